# revision 28
# baseline (speedup 1.0000x reference)
"""Trainium2 Bass kernel for nn_ConvolutionAttention (v3: bf16+fp8DR pipeline).

Reference computation (per batch element b of B=8):
  x1 = features1[b] as [C=256, 32, 32];  x2 = features2[b] likewise
  q = pw(bn(dw3x3(x1)));  k = pw(bn(dw3x3(x2)));  v same as k w/ own weights
  per head h (8 heads, dh=64): attn = softmax(q_h k_h^T / 8);  o_h = attn v_h
  out[b] = concat_h(o_h) @ ffn_w.T + ffn_b      -> [1024, 256]

Sharding: pure data-parallel over batch; core i computes batch element i.

Design notes:
  - all matmul operands bf16 (f32r streams at half rate on HW); q/k depthwise
    conv uses fp8 DoubleRow (two taps contracted per matmul via overlapping-
    window APs), with a x16 weight scale folded out through the EXP scale.
  - ACT-engine EXP (64 x ~1.2us, serial) is the pacing floor: scores for head
    order[i+2] are jb-interleaved with av for head order[i]; v-path convs fill
    the EXP-paced h0/h1 stretch.
  - v pointwise bias folded through ffn_w into ffn_b on host (exact); k/q
    biases via DVE tensor_scalar; ffn bias via DVE add of a replicated tile.
  - one dma_start per logical input (the Sync queue serializes triggers at
    ~0.65us each); vt ones column via gpsimd memset.
  - tail: sp pool closes after the last scores so 4 PSUM banks pre-run ffn
    kc0-2 for nb0-3; ffn kc3 split into per-head K=64 halves so the h7 half
    issues right after norm7.
"""

import numpy as np

import concourse.bass as bass
import concourse.bacc as bacc
import concourse.tile as tile
from concourse import mybir
from concourse.bass_utils import run_bass_kernel_spmd

F32 = mybir.dt.float32
BF16 = mybir.dt.bfloat16
FP8 = mybir.dt.float8e4

B, C, HWN, H, W = 8, 256, 1024, 32, 32
HEADS, DH, OC = 8, 64, 512
SCALE = DH ** -0.5
EPS = 1e-5
PAD = 34 * 34  # 1156

# fp8 DoubleRow tap pairing for the 3x3 depthwise conv: pairs with constant
# window offset delta, plus the leftover 9th tap
DR_PAIRS = [(0, 1), (3, 4), (6, 7), (2, 5)]

_CACHE = {}


# ----------------------------------------------------------------- device code

def _emit(nc, tc):
    dt = nc.dram_tensor
    d = {
        "xq8": dt("xq8", [128, 2 * PAD], FP8, kind="ExternalInput").ap(),
        "dwq8": dt("dwq8", [128, 2304], FP8, kind="ExternalInput").ap(),
        "xkv8": dt("xkv8", [128, 2 * PAD], FP8, kind="ExternalInput").ap(),
        "dwk8": dt("dwk8", [128, 2304], FP8, kind="ExternalInput").ap(),
        "wqkv": dt("wqkv", [128, 3072], BF16, kind="ExternalInput").ap(),
        "xkv": dt("xkv", [128, 2 * PAD], BF16, kind="ExternalInput").ap(),
        "eye": dt("eye", [128, 128], BF16, kind="ExternalInput").ap(),
        "dwt": dt("dwt", [128, 54], BF16, kind="ExternalInput").ap(),
        "qk_bias": dt("qk_bias", [128, 8], F32, kind="ExternalInput").ap(),
        "ones_bc": dt("ones_bc", [1, 64], BF16, kind="ExternalInput").ap(),
        "ffnw": dt("ffnw", [128, 1024], BF16, kind="ExternalInput").ap(),
        "ffnb": dt("ffnb", [128, 2], F32, kind="ExternalInput").ap(),
        "out": dt("out", [C, HWN], F32, kind="ExternalOutput").ap(),
    }
    with nc.allow_low_precision(reason="bf16/fp8 matmul pipeline"):
        _emit_body(nc, tc, d)


def _emit_body(nc, tc, d):
    import bass_rust
    from contextlib import ExitStack
    mm = nc.tensor.matmul

    with tc.tile_pool(name="const", bufs=1) as const:
        # fp8 q/k-path inputs first: they gate the first matmul; one dma_start
        # per tensor (each trigger costs ~0.65us on the Sync queue)
        x8q = const.tile([128, 2 * PAD], FP8, tag="x8q", name="x8q")
        nc.sync.dma_start(x8q[:, 0:PAD], d["xq8"][:, 0:PAD])
        dw8q = const.tile([128, 2304], FP8, tag="dw8q", name="dw8q")
        nc.sync.dma_start(dw8q[:, 0:1152], d["dwq8"][:, 0:1152])
        nc.sync.dma_start(x8q[:, PAD:2 * PAD], d["xq8"][:, PAD:2 * PAD])
        nc.sync.dma_start(dw8q[:, 1152:2304], d["dwq8"][:, 1152:2304])
        x8kv = const.tile([128, 2 * PAD], FP8, tag="x8kv", name="x8kv")
        nc.sync.dma_start(x8kv[:], d["xkv8"])
        dw8k = const.tile([128, 2304], FP8, tag="dw8k", name="dw8k")
        nc.sync.dma_start(dw8k[:], d["dwk8"])
        eye_sb = const.tile([128, 128], BF16, tag="eye", name="eye_sb")
        nc.sync.dma_start(eye_sb[:], d["eye"])
        dwt_sb = const.tile([128, 54], BF16, tag="dwt", name="dwt_sb")
        nc.sync.dma_start(dwt_sb[:], d["dwt"])
        xkv_sb = const.tile([128, 2 * PAD], BF16, tag="xkv", name="xkv_sb")
        nc.sync.dma_start(xkv_sb[:, 0:PAD], d["xkv"][:, 0:PAD])
        wqkv_sb = const.tile([128, 3072], BF16, tag="wqkv", name="wqkv_sb")
        nc.sync.dma_start(wqkv_sb[:], d["wqkv"])
        nc.sync.dma_start(xkv_sb[:, PAD:2 * PAD], d["xkv"][:, PAD:2 * PAD])
        qkb_sb = const.tile([128, 8], F32, tag="qkb", name="qkb")
        nc.sync.dma_start(qkb_sb[:], d["qk_bias"])
        ones_bc_sb = const.tile([1, 64], BF16, tag="onesbc", name="ones_bc_sb")
        nc.sync.dma_start(ones_bc_sb[:], d["ones_bc"])
        ffnw_sb = const.tile([128, 1024], BF16, tag="ffnw", name="ffnw_sb")
        nc.sync.dma_start(ffnw_sb[:], d["ffnw"])
        ffnb_sb = const.tile([128, 2], F32, tag="ffnb", name="ffnbsb")
        nc.sync.dma_start(ffnb_sb[:], d["ffnb"])

        def wsl(p, kc):      # bf16 pointwise weight slice [128, 512]
            i = {"q": 0, "k": 1, "v": 2}[p] * 2 + kc
            return wqkv_sb[:, i * 512:(i + 1) * 512]

        def fwsl(kc):        # ffn weight slice [128, 256]
            return ffnw_sb[:, kc * 256:(kc + 1) * 256]

        # persistent activations (bf16)
        y_sb = {}   # dw conv outputs, filled in pipeline order
        q_sb = [const.tile([128, HWN], BF16, tag=f"qsb{i}", name=f"qsb{i}") for i in range(4)]
        k_sb = [const.tile([128, HWN], BF16, tag=f"ksb{i}", name=f"ksb{i}") for i in range(4)]
        vt_sb = [const.tile([128, 8 * 66], BF16, tag=f"vt{i}", name=f"vt{i}") for i in range(8)]
        ot_sb = [const.tile([128, HWN], BF16, tag=f"ot{i}", name=f"ot{i}") for i in range(4)]
        for i in range(8):
            vtv = vt_sb[i][:].rearrange("p (h c) -> p h c", c=66)
            nc.gpsimd.memset(vtv[:, :, 64:65], 1.0)

        # dw diag construction on DVE (v path only; q,k ship fp8 from host)
        dwd_sb = {}
        for blk in range(2):
            t = const.tile([128, 9 * 128], BF16, tag=f"dwv{blk}", name=f"dwdv{blk}")
            i0 = 2 * 18 + blk * 9
            e3 = eye_sb[:].rearrange("p (a c) -> p a c", a=1)
            w3 = dwt_sb[:, i0:i0 + 9].rearrange("p (a c) -> p a c", c=1)
            e3b, w3b = bass.broadcast_tensor_aps(e3, w3)
            nc.vector.tensor_tensor(
                t[:].rearrange("p (a c) -> p a c", c=128), e3b, w3b,
                op=mybir.AluOpType.mult)
            dwd_sb["v", blk] = t

        # ---------------- fused conv + attention pipeline ----------------
        order = [0, 1, 2, 3, 4, 5, 7, 6]
        with tc.tile_pool(name="epool", bufs=24) as epool, \
             tc.tile_pool(name="norm", bufs=2) as npool:
            e_tiles = {}
            cur_sp = [None]   # active scores PSUM pool (phase A then phase B)

            def emit_scores_jb(h, jb):
                pair, pb = h // 2, (h % 2) * 64
                sp = cur_sp[0].tile([128, HWN], F32, tag="sp", name="sp")
                for hf in range(2):
                    mm(sp[:, hf * 512:(hf + 1) * 512],
                       k_sb[pair][pb:pb + 64, jb * 128:(jb + 1) * 128],
                       q_sb[pair][pb:pb + 64, hf * 512:(hf + 1) * 512],
                       start=True, stop=True)
                e = epool.tile([128, HWN], BF16, tag="e", name="e")
                nc.scalar.activation(e[:], sp[:],
                                     mybir.ActivationFunctionType.Exp,
                                     scale=SCALE / 256.0)
                e_tiles[h, jb] = e

            def emit_av_jb(h, jb, oacc):
                for hf in range(2):
                    mm(oacc[:, hf * 512:(hf + 1) * 512],
                       vt_sb[jb][:, 66 * h: 66 * h + 65],
                       e_tiles[h, jb][:, hf * 512:(hf + 1) * 512],
                       start=(jb == 0), stop=(jb == 7))

            def emit_norm(h, oacc, opool, tag):
                pair, pb = h // 2, (h % 2) * 64
                o_un = npool.tile([65, HWN], F32, tag="oun", name="o_un")
                if h in (5, 7, 6):
                    # tail heads: ACT is idle once the EXP stream drains, and
                    # the DVE FIFO would delay this copy behind conv copies
                    nc.scalar.copy(o_un[:], oacc[:])
                else:
                    nc.vector.tensor_copy(o_un[:], oacc[:])
                # reciprocal of the denominator row: reshape to [64,16] via DMA
                # (a [1,1024] single-lane DVE reciprocal measures 6.5us)
                csp = npool.tile([64, 16], F32, tag="csp", name="csp")
                nc.sync.dma_start(
                    csp[:], o_un[64:65, :].rearrange("p (a b) -> p a b", b=16))
                csr = npool.tile([64, 16], BF16, tag="csr", name="csr")
                nc.vector.reciprocal(csr[:], csp[:])
                rrow = npool.tile([1, HWN], BF16, tag="rrow", name="rrow")
                nc.sync.dma_start(
                    rrow[:].rearrange("p (a b) -> p a b", b=16), csr[:])
                # rank-1 broadcast of rrow across 64 partitions (reuses oacc bank)
                bc = opool.tile([64, HWN], F32, tag=tag, name=f"bc{h}")
                for hf in range(2):
                    mm(bc[:, hf * 512:(hf + 1) * 512], ones_bc_sb[0:1, :],
                       rrow[0:1, hf * 512:(hf + 1) * 512], start=True, stop=True)
                if pb == 0:
                    nc.vector.tensor_mul(ot_sb[pair][0:64, :], o_un[0:64, :], bc[:])
                else:
                    # lanes can't cross partitions: normalize at base 0,
                    # then DMA the finished bf16 slice up to partitions 64:128
                    tmp = npool.tile([64, HWN], BF16, tag="otmp", name="ot_tmp")
                    nc.vector.tensor_mul(tmp[:], o_un[0:64, :], bc[:])
                    nc.sync.dma_start(ot_sb[pair][64:128, :], tmp[:])

            def gen_dw_dr(p, blk, pool):
                """fp8 DoubleRow depthwise conv: 2 taps contracted per matmul
                via overlapping-window access patterns."""
                ps = pool.tile([128, HWN], F32, tag="ps", name=f"ps8{p}{blk}")
                xt = x8q if p == "q" else x8kv
                xv = xt[:, blk * PAD:(blk + 1) * PAD].rearrange(
                    "p (r c) -> p r c", c=34)
                pstride = x8q[:].ap[0][0]
                dw8 = dw8q if p == "q" else dw8k
                for pi, (t1, t2) in enumerate(DR_PAIRS):
                    d1, j1 = t1 // 3, t1 % 3
                    d2, j2 = t2 // 3, t2 % 3
                    delta = (d2 - d1) * 34 + (j2 - j1)
                    lhsT = dw8[:, blk * 1152 + 256 * pi: blk * 1152 + 256 * pi + 256
                               ].rearrange("p (a c) -> p a c", c=128)
                    for hf in range(2):
                        rhs = xv[:, d1 + hf * 16: d1 + hf * 16 + 16,
                                 j1: j1 + 32].unsqueeze(1).broadcast_to(
                                     [128, 2, 16, 32])
                        rhs.ap = bass_rust.VecI64Pair(
                            [[pstride, 128], [delta, 2], [34, 16], [1, 32]])
                        mm(ps[:, hf * 512:(hf + 1) * 512], lhsT, rhs,
                           start=(pi == 0), stop=False,
                           perf_mode=mybir.MatmulPerfMode.DoubleRow)
                    yield
                lhsT8 = dw8[:, blk * 1152 + 1024: blk * 1152 + 1152]
                for hf in range(2):
                    rhs = xv[:, 2 + hf * 16: 2 + hf * 16 + 16, 2:34]
                    mm(ps[:, hf * 512:(hf + 1) * 512], lhsT8, rhs,
                       start=False, stop=True)
                yield
                y = const.tile([128, HWN], BF16, tag=f"y{p}{blk}", name=f"y{p}{blk}")
                nc.vector.tensor_copy(y[:], ps[:])
                y_sb[p, blk] = y

            def gen_dw_v(blk, pool):
                ps = pool.tile([128, HWN], F32, tag="ps", name=f"psdwv{blk}")
                xv = xkv_sb[:, blk * PAD:(blk + 1) * PAD].rearrange(
                    "p (r c) -> p r c", c=34)
                for tap in range(9):
                    di, dj = tap // 3, tap % 3
                    lhsT = dwd_sb["v", blk][:, tap * 128:(tap + 1) * 128]
                    for hf in range(2):
                        rhs = xv[:, di + hf * 16: di + hf * 16 + 16, dj: dj + 32]
                        mm(ps[:, hf * 512:(hf + 1) * 512], lhsT, rhs,
                           start=(tap == 0), stop=(tap == 8))
                    yield
                y = const.tile([128, HWN], BF16, tag="yv" + str(blk), name=f"yv{blk}")
                nc.vector.tensor_copy(y[:], ps[:])
                y_sb["v", blk] = y

            def gen_pw_qk(pair, pool):
                for p, dest in (("q", q_sb), ("k", k_sb)):
                    ps = pool.tile([128, HWN], F32, tag="ps", name=f"pspw{p}{pair}")
                    for kc in range(2):
                        for hf in range(2):
                            mm(ps[:, hf * 512:(hf + 1) * 512],
                               wsl(p, kc)[:, pair * 128:(pair + 1) * 128],
                               y_sb[p, kc][:, hf * 512:(hf + 1) * 512],
                               start=(kc == 0), stop=(kc == 1))
                        yield
                    ci = 0 if p == "q" else 1
                    nc.vector.tensor_scalar_add(
                        dest[pair][:], ps[:],
                        qkb_sb[:, ci * 4 + pair: ci * 4 + pair + 1])

            def gen_pw_v(half, pool):
                ps = pool.tile([128, HWN], F32, tag="ps", name=f"psv{half}")
                for sub in range(2):
                    mb = 2 * half + sub
                    for kc in range(2):
                        mm(ps[:, sub * 512:(sub + 1) * 512],
                           y_sb["v", kc][:, mb * 128:(mb + 1) * 128],
                           wsl("v", kc), start=(kc == 0), stop=(kc == 1))
                    yield
                for sub in range(2):
                    mb = 2 * half + sub
                    vtv = vt_sb[mb][:].rearrange("p (h c) -> p h c", c=66)
                    nc.vector.tensor_copy(vtv[:, :, 0:64],
                                          ps[:, sub * 512:(sub + 1) * 512])

            # --- conv stage + EXP-paced scores for h0,h1 ---
            stPS = ExitStack()
            cur_sp[0] = stPS.enter_context(
                tc.tile_pool(name="sp", bufs=2, space="PSUM"))
            with tc.tile_pool(name="psdw", bufs=2, space="PSUM") as psdw:
                for blk in range(2):
                    for _ in gen_dw_dr("q", blk, psdw):
                        pass
                for blk in range(2):
                    for _ in gen_dw_dr("k", blk, psdw):
                        pass
                for _ in gen_pw_qk(0, psdw):
                    pass
                # v path + remaining qk pointwise, stepped as fillers between
                # the EXP-paced h0/h1 scores
                pending = [gen_dw_v(0, psdw), gen_pw_qk(1, psdw),
                           gen_dw_v(1, psdw)] + \
                          [gen_pw_v(half, psdw) for half in range(4)] + \
                          [gen_pw_qk(2, psdw), gen_pw_qk(3, psdw)]

                def step_fill(n):
                    while n > 0 and pending:
                        try:
                            next(pending[0])
                            n -= 1
                        except StopIteration:
                            pending.pop(0)

                for h in order[:2]:
                    for jb in range(8):
                        emit_scores_jb(h, jb)
                        step_fill(2)
                step_fill(10 ** 6)

            # --- attention stage ---
            stA = ExitStack()
            with tc.tile_pool(name="oaccB", bufs=1, space="PSUM") as opB:
                opA = stA.enter_context(
                    tc.tile_pool(name="oaccA", bufs=1, space="PSUM"))
                op = {0: (opA, "oA"), 1: (opB, "oB")}
                for idx in range(6):           # heads 0..5
                    h = order[idx]
                    opool, tag = op[idx % 2]
                    oacc = opool.tile([65, HWN], F32, tag=tag, name=f"oacc{h}")
                    h2 = order[idx + 2]
                    for jb in range(8):
                        emit_scores_jb(h2, jb)
                        emit_av_jb(h, jb, oacc)
                    emit_norm(h, oacc, opool, tag)
                # last two heads back to back; their normalize chains
                # (DVE + DMA latency) hide under the ffn / av matmuls
                oacc7 = opA.tile([65, HWN], F32, tag="oA", name="oacc7")
                for jb in range(8):
                    emit_av_jb(7, jb, oacc7)
                oacc6 = opB.tile([65, HWN], F32, tag="oB", name="oacc6")
                for jb in range(8):
                    emit_av_jb(6, jb, oacc6)
                emit_norm(7, oacc7, opA, "oA")
                stA.close()   # frees 2 banks for the early ffn blocks
                with tc.tile_pool(name="psf", bufs=1, space="PSUM") as psf:
                    # transposed ffn: out.T[c, hw]; tiles [128 c, 512 hw] so
                    # matmuls run N=512 with LDWEIGHTS hidden; one open
                    # accumulation group per 2KB PSUM bank
                    f_t = [psf.tile([128, 512], F32, tag=f"f{i}", name=f"psf{i}")
                           for i in range(2)]
                    # pre-issue kc 0..2 for (c0, hw0) and (c1, hw0)
                    for ci in range(2):
                        for kc in range(3):
                            mm(f_t[ci][:], fwsl(kc)[:, ci * 128:(ci + 1) * 128],
                               ot_sb[kc][:, 0:512], start=(kc == 0), stop=False)
                    # h7's half of kc3 (K=64, partitions 64:128) right away
                    for ci in range(2):
                        mm(f_t[ci][:], fwsl(3)[64:128, ci * 128:(ci + 1) * 128],
                           ot_sb[3][64:128, 0:512], start=False, stop=False)
                    emit_norm(6, oacc6, opB, "oB")
                    with tc.tile_pool(name="fout", bufs=4) as fpool:
                        def fin(ci, hw, t):
                            fo = fpool.tile([128, 512], F32, tag="fo", name="fo")
                            nc.vector.tensor_scalar_add(
                                fo[:], t[:], ffnb_sb[:, ci:ci + 1])
                            nc.sync.dma_start(
                                d["out"][ci * 128:(ci + 1) * 128,
                                         hw * 512:(hw + 1) * 512], fo[:])
                        for ci in range(2):   # h6's half of kc3, finish hw0
                            mm(f_t[ci][:], fwsl(3)[0:64, ci * 128:(ci + 1) * 128],
                               ot_sb[3][0:64, 0:512], start=False, stop=True)
                            fin(ci, 0, f_t[ci])
                        for ci in range(2):   # hw1 blocks, reuse banks
                            for kc in range(4):
                                mm(f_t[ci][:], fwsl(kc)[:, ci * 128:(ci + 1) * 128],
                                   ot_sb[kc][:, 512:1024], start=(kc == 0),
                                   stop=(kc == 3))
                            fin(ci, 1, f_t[ci])
            stPS.close()


def _build():
    nc = bacc.Bacc("TRN2", target_bir_lowering=False, debug=False)
    with tile.TileContext(nc) as tc:
        _emit(nc, tc)
    nc.compile()
    return nc


# ----------------------------------------------------------------- host code

def _host_shared(inputs):
    import ml_dtypes
    g = lambda n: np.asarray(inputs[n], dtype=np.float32)
    d = {}
    dw_effs = []
    qk_bias_cols = []
    pws = []
    for ci, p in enumerate(("q", "k", "v")):
        a = g(f"{p}_bn_g") / np.sqrt(g(f"{p}_bn_v") + EPS)          # [256]
        dw_eff = g(f"{p}_dw_w")[:, 0] * a[:, None, None]            # [256,3,3]
        beta = a * g(f"{p}_dw_b") + g(f"{p}_bn_b") - a * g(f"{p}_bn_m")
        pw = g(f"{p}_pw_w")[:, :, 0, 0]                             # [512,256]
        bias = g(f"{p}_pw_b") + pw @ beta                           # [512]
        dw_effs.append(dw_eff)
        pws.append(np.ascontiguousarray(pw.T))                      # [256,512]
        if p == "v":
            vbias = bias                                            # [512]
        else:
            qk_bias_cols.append(bias)
    # pointwise weights, one fused tensor: [128, (q0,q1,k0,k1,v0,v1) x 512]
    wqkv = np.zeros((128, 3072), np.float32)
    for pi in range(3):
        for kc in range(2):
            wqkv[:, (pi * 2 + kc) * 512:(pi * 2 + kc + 1) * 512] = \
                pws[pi][kc * 128:(kc + 1) * 128]
    d["wqkv"] = wqkv
    # q,k biases carry the x16 of the fp8 depthwise weights
    qkb = np.zeros((128, 8), np.float32)
    for ci in range(2):
        for mb in range(4):
            qkb[:, ci * 4 + mb] = 16.0 * qk_bias_cols[ci][mb * 128:(mb + 1) * 128]
    d["qk_bias"] = qkb
    # fp8 DoubleRow diag weights for q,k (x16, folded out via EXP scale /256)
    for nm, ci in (("dwq8", 0), ("dwk8", 1)):
        w = dw_effs[ci].reshape(256, 9) * 16.0
        m = np.zeros((128, 2304), np.float32)
        for blk in range(2):
            wb = w[blk * 128:(blk + 1) * 128]
            o = blk * 1152
            for pi, (t1, t2) in enumerate(DR_PAIRS):
                m[:, o + 256 * pi: o + 256 * pi + 128] = np.diag(wb[:, t1])
                m[:, o + 256 * pi + 128: o + 256 * pi + 256] = np.diag(wb[:, t2])
            m[:, o + 1024: o + 1152] = np.diag(wb[:, 8])
        d[nm] = m.astype(ml_dtypes.float8_e4m3fn)
    d["eye"] = np.eye(128, dtype=np.float32)
    dwtm = np.zeros((128, 54), np.float32)
    for ci in range(3):
        for blk in range(2):
            for t in range(9):
                dwtm[:, ci * 18 + blk * 9 + t] = dw_effs[ci][blk * 128:(blk + 1) * 128, t // 3, t % 3]
    d["dwt"] = dwtm
    d["ones_bc"] = np.ones((1, 64), np.float32)
    # ffn weights [128, 4x256]; v pointwise bias folded through ffn (exact)
    fw = np.ascontiguousarray(g("ffn_w").T)                         # [512, 256]
    ffnw = np.zeros((128, 1024), np.float32)
    for kc in range(4):
        ffnw[:, kc * 256:(kc + 1) * 256] = fw[kc * 128:(kc + 1) * 128]
    d["ffnw"] = ffnw
    ffnb_eff = g("ffn_b") + g("ffn_w") @ vbias                      # [256]
    d["ffnb"] = np.ascontiguousarray(ffnb_eff.reshape(2, 128).T)    # [128, 2]
    for k in ("wqkv", "eye", "dwt", "ones_bc", "ffnw"):
        d[k] = d[k].astype(ml_dtypes.bfloat16)
    return d


def _host_x(feat):
    # [1024, 256] -> padded transposed [128, 2 x 34*34]
    xt = np.ascontiguousarray(np.asarray(feat, np.float32).T).reshape(2, 128, 32, 32)
    xp = np.zeros((2, 128, 34, 34), np.float32)
    xp[:, :, 1:33, 1:33] = xt
    return np.ascontiguousarray(
        xp.reshape(2, 128, PAD).transpose(1, 0, 2)).reshape(128, 2 * PAD)


def make_in_maps(inputs):
    import ml_dtypes
    shared = _host_shared(inputs)
    f1 = np.asarray(inputs["features1"], dtype=np.float32)
    f2 = np.asarray(inputs["features2"], dtype=np.float32)
    maps = []
    for b in range(B):
        m = dict(shared)
        x1 = _host_x(f1[b])
        x2 = _host_x(f2[b])
        m["xq8"] = x1.astype(ml_dtypes.float8_e4m3fn)
        m["xkv8"] = x2.astype(ml_dtypes.float8_e4m3fn)
        m["xkv"] = x2.astype(ml_dtypes.bfloat16)
        maps.append(m)
    return maps


def get_nc():
    if "nc" not in _CACHE:
        _CACHE["nc"] = _build()
    return _CACHE["nc"]


def kernel(**inputs):
    nc = get_nc()
    in_maps = make_in_maps(inputs)
    res = run_bass_kernel_spmd(nc, in_maps, list(range(B)))
    return np.stack([res.results[i]["out"].T for i in range(B)]).astype(np.float32)
